# revision 41
# baseline (speedup 1.0000x reference)
"""Trainium2 Bass kernel for AttentionMM.

Reference computation (per batch b, T=E=512):
    alpha = softmax(x1 @ x2^T, axis=-1)              # [T, T]
    a1t   = alpha^T @ x2                             # [T, E]
    a2t   = alpha @ x1                               # [T, E]
    o1    = mean_t tanh(x1 @ U1 + a1t @ V1)          # [E]
    o2    = mean_t tanh(x2 @ U2 + a2t @ V2)          # [E]
    out[b] = concat(o1, o2)                          # [2E]

Sharding: data-parallel over batch across 8 NeuronCores (4 batches/core),
U1/U2/V1/V2 replicated.  No collectives needed; shard/gather on host.

Matmuls run in fp16 (full TensorEngine rate, ~8x better accuracy than
bf16) except a1's, which runs in fp8 e4m3 with DoubleRow (two 128-deep
k-tiles per matmul; alpha quantizes well, measured rel_err 1.47e-2 vs the
2e-2 gate, emulator-validated).  Accumulation is f32 in PSUM.  Per-batch
dataflow keeps everything in "transposed" layouts so that every
contraction lands on the partition axis and the final mean over T is a
free-axis reduction (done for free by activation(accum_out=...)):
    S      = x1 @ x2^T          via lhsT=x1^T blocks, rhs=x2^T   -> [t, s]
    alpha  = softmax rows (reduce_max(negate) -> Exp(bias=-max, accum_out)
             -> reciprocal -> tensor_scalar_mul)
    alphaT = PE-transpose of alpha blocks (16x 128x128)
    a1^T   = lhsT=x2 blocks,  rhs=alpha                          -> [e, t]
    a2^T   = lhsT=x1 blocks,  rhs=alphaT                         -> [e, t]
    o1pre^T= lhsT=U1 blocks, rhs=x1^T  (+) lhsT=V1 blocks, rhs=a1^T
    o2pre^T= lhsT=U2 blocks, rhs=x2^T  (+) lhsT=V2 blocks, rhs=a2^T
    tanh with accum_out -> per-partition sums -> staged, one final PE
    transpose + scale by 1/T -> single contiguous DMA out.

PSUM budget (8 banks): 3 for the S chains (i-outer, stall-free vs the
softmax consumer chain), 3 shared ring for alpha-transposes + a1/a2, 2 for
the o-phase.  PE warmups run on a zeroed tile with no data dependencies so
the HAM clock-gate lifts before the first real matmul.
"""

import sys

if "/opt/trn_rl_repo" not in sys.path:
    sys.path.insert(0, "/opt/trn_rl_repo")

import numpy as np

B, T, E = 32, 512, 512
NCORES = 8
BL = B // NCORES  # batches per core
P = 128
NT = T // P
NE = E // P

_CACHE = {}


def _build():
    from contextlib import ExitStack

    import concourse.bass as bass
    import concourse.tile as tile
    from concourse import bacc, mybir
    from concourse.masks import make_identity

    f16 = mybir.dt.float16
    f32 = mybir.dt.float32
    f8 = mybir.dt.float8e4
    DR = mybir.MatmulPerfMode.DoubleRow
    AF = mybir.ActivationFunctionType
    AX = mybir.AxisListType

    nc = bacc.Bacc(
        "TRN2",
        target_bir_lowering=False,
        debug=False,
        enable_asserts=False,
        num_devices=NCORES,
    )

    x1_d = nc.dram_tensor("x1", [BL, T, E], f16, kind="ExternalInput")
    # x2 in row-major layout only feeds a1's fp8 DoubleRow lhsT
    x2_d = nc.dram_tensor("x2", [BL, T, E], f8, kind="ExternalInput")
    # host-pretransposed copies: x1t[b, e, t] = x1[b, t, e]
    x1t_d = nc.dram_tensor("x1t", [BL, E, T], f16, kind="ExternalInput")
    x2t_d = nc.dram_tensor("x2t", [BL, E, T], f16, kind="ExternalInput")
    w_d = {
        nm: nc.dram_tensor(nm, [E, E], f16, kind="ExternalInput")
        for nm in ("u1", "v1", "u2", "v2")
    }
    out_d = nc.dram_tensor("out", [BL, 2 * E], f32, kind="ExternalOutput")

    with tile.TileContext(nc) as tc, ExitStack() as ctx:
        const = ctx.enter_context(tc.tile_pool(name="const", bufs=1))
        wpool = ctx.enter_context(tc.tile_pool(name="wts", bufs=1))
        xpool = ctx.enter_context(tc.tile_pool(name="x", bufs=BL))
        apool = ctx.enter_context(tc.tile_pool(name="alpha", bufs=2))
        cpool = ctx.enter_context(tc.tile_pool(name="attn", bufs=2))
        spool = ctx.enter_context(tc.tile_pool(name="stats", bufs=16))
        tpool = ctx.enter_context(tc.tile_pool(name="trash", bufs=2))
        stgp = ctx.enter_context(tc.tile_pool(name="stage", bufs=1))
        ps_s = ctx.enter_context(tc.tile_pool(name="ps_s", bufs=3, space="PSUM"))
        ps_ta = ctx.enter_context(tc.tile_pool(name="ps_ta", bufs=3, space="PSUM"))
        ps_o = ctx.enter_context(tc.tile_pool(name="ps_o", bufs=2, space="PSUM"))

        # Warmup operand: memset on the vector engine (whose preamble ends
        # earliest) so the warmup matmuls issue as soon as possible and
        # lift the HAM clock gate before the first data-dependent matmul.
        warm = const.tile([P, T], f16, tag="warm")
        nc.vector.memset(warm[:], 0)
        id_f16 = const.tile([P, P], f16, tag="id_f16")
        make_identity(nc, id_f16[:])
        # identity pre-scaled by 1/T: the final stage transpose then yields
        # the mean directly (one DVE copy instead of a scalar mul after)
        id_f32s = const.tile([P, P], f32, tag="id_f32s")
        make_identity(nc, id_f32s[:])
        nc.scalar.mul(id_f32s[:], id_f32s[:], 1.0 / T)

        # col = b*8 + half*4 + f  ->  out[b, half*512 + f*128 : +128]
        stage = stgp.tile([P, 8 * BL], f32, tag="stage")

        # ---- tiles (allocated up front so DMAs can be emitted in the
        # exact order they should occupy the single sync DMA ring) ----
        x1t = {}
        x2t = {}
        x1n = {}
        x2n = {}
        for b in range(BL):
            x1t[b] = xpool.tile([P, NE, T], f16, tag="x1t", name=f"x1t{b}")
            x2t[b] = xpool.tile([P, NE, T], f16, tag="x2t", name=f"x2t{b}")
            x1n[b] = xpool.tile([P, NT, E], f16, tag="x1n", name=f"x1n{b}")
            x2n[b] = xpool.tile([P, NT, E], f8, tag="x2n", name=f"x2n{b}")
        ws = {
            nm: wpool.tile([P, NE, E], f16, tag=nm, name=nm)
            for nm in ("u1", "v1", "u2", "v2")
        }

        def load_xt(b, chunked=False):
            if chunked:
                # interleave half-tensor slabs so the first S accumulation
                # chain can start after the first pair arrives (same-tile
                # chunk DMAs serialize on their completion semaphore, so
                # keep the chunk count low)
                H = NE // 2
                for h in range(2):
                    nc.sync.dma_start(
                        x1t[b][:, h * H : (h + 1) * H, :],
                        x1t_d.ap()[b, h * H * P : (h + 1) * H * P, :].rearrange(
                            "(a p) t -> p a t", p=P
                        ),
                    )
                    nc.sync.dma_start(
                        x2t[b][:, h * H : (h + 1) * H, :],
                        x2t_d.ap()[b, h * H * P : (h + 1) * H * P, :].rearrange(
                            "(a p) t -> p a t", p=P
                        ),
                    )
            else:
                nc.sync.dma_start(
                    x1t[b][:], x1t_d.ap()[b].rearrange("(a p) t -> p a t", p=P)
                )
                nc.sync.dma_start(
                    x2t[b][:], x2t_d.ap()[b].rearrange("(a p) t -> p a t", p=P)
                )

        def load_xn(b):
            # x2n first: it is a1's lhsT, the first consumer after softmax
            nc.sync.dma_start(
                x2n[b][:], x2_d.ap()[b].rearrange("(i p) e -> p i e", p=P)
            )
            nc.sync.dma_start(
                x1n[b][:], x1_d.ap()[b].rearrange("(i p) e -> p i e", p=P)
            )

        def load_w(nm):
            nc.sync.dma_start(ws[nm][:], w_d[nm].ap().rearrange("(a p) f -> p a f", p=P))

        # All loads on ONE queue (sync), in strict need-time order: a single
        # FIFO ring keeps early consumers' transfers from being bandwidth-
        # starved by later ones.
        load_xt(0, chunked=True)
        load_xt(1, chunked=True)
        load_xn(0)
        for nm in ("u1", "v1", "u2", "v2"):
            load_w(nm)
        load_xt(2)
        load_xn(1)
        load_xt(3)
        load_xn(2)
        load_xn(3)

        # PE warmups (HAM clock gate): no data deps, results discarded.
        # They rotate through the o-phase PSUM ring, which has no real
        # consumer until ~18us in, so they never delay the S chains.
        for k in range(8):
            warm_ps = ps_o.tile([P, T], f32, tag="o", name=f"warm{k}")
            nc.tensor.matmul(
                warm_ps[:], lhsT=warm[:, :P], rhs=warm[:], start=True, stop=True
            )

        def s_phase(b):
            """S = x1 @ x2^T, then row softmax -> alpha [t-part, s-free].
            Also materializes an fp8 copy of alpha for a1's DoubleRow mms."""
            alpha = apool.tile([P, NT, T], f16, tag="alpha")
            alpha8 = apool.tile([P, NT, T], f8, tag="alpha8")
            expt = apool.tile([P, NT, T], f16, tag="expt")
            for i in range(NT):
                ps = ps_s.tile([P, T], f32, tag="s")
                for e in range(NE):
                    nc.tensor.matmul(
                        ps[:],
                        lhsT=x1t[b][:, e, i * P : (i + 1) * P],
                        rhs=x2t[b][:, e, :],
                        start=(e == 0),
                        stop=(e == NE - 1),
                    )
                mneg = spool.tile([P, 1], f32, tag="mneg")
                nc.vector.reduce_max(out=mneg[:], in_=ps[:], axis=AX.X, negate=True)
                ssum = spool.tile([P, 1], f32, tag="ssum")
                nc.scalar.activation(
                    expt[:, i, :], ps[:], AF.Exp, bias=mneg[:], accum_out=ssum[:]
                )
                rcol = spool.tile([P, 1], f32, tag="rcol")
                nc.vector.reciprocal(rcol[:], ssum[:])
                nc.vector.tensor_scalar_mul(alpha[:, i, :], expt[:, i, :], rcol[:])
                # fp8 copy on the SCALAR engine (normalize via the per-
                # partition scale operand): keeps alpha8 off the DVE FIFO,
                # where it would queue behind the next batch's softmax stats
                # and stall a1's DoubleRow weight loads.  (GpSimd is 4.4x
                # slower at this cast; DVE ordering stalls a1 by ~3us.)
                nc.scalar.activation(
                    alpha8[:, i, :], expt[:, i, :], AF.Copy, scale=rcol[:]
                )
            return alpha, alpha8

        def transpose_alpha(alpha):
            """alphaT[j-part, t-free] via 16 PE block transposes, emitted
            ahead of the NEXT batch's S matmuls."""
            alphaT = apool.tile([P, NT, T], f16, tag="alphaT")
            for j in range(NT):
                pst = ps_ta.tile([P, T], f16, tag="ta", name="pst")
                for i in range(NT):
                    nc.tensor.transpose(
                        pst[:, i * P : (i + 1) * P],
                        alpha[:, i, j * P : (j + 1) * P],
                        id_f16[:],
                    )
                nc.vector.tensor_copy(out=alphaT[:, j, :], in_=pst[:])
            return alphaT

        def rest_phase(b, alpha8, alphaT):
            # a1^T[e, t] = sum_k x2[k, e] * alpha[k, t]  -- fp8 DoubleRow:
            # each matmul contracts two 128-deep k-tiles at once.
            a1 = cpool.tile([P, NE, T], f16, tag="a1")
            for e in range(NE):
                pa = ps_ta.tile([P, T], f32, tag="ta", name="pa1")
                for q in range(NT // 2):
                    nc.tensor.matmul(
                        pa[:],
                        lhsT=x2n[b][:, 2 * q : 2 * q + 2, e * P : (e + 1) * P],
                        rhs=alpha8[:, 2 * q : 2 * q + 2, :],
                        start=(q == 0),
                        stop=(q == NT // 2 - 1),
                        perf_mode=DR,
                    )
                nc.vector.tensor_copy(out=a1[:, e, :], in_=pa[:])
            # a2^T[e, t] = sum_s x1[s, e] * alphaT[s, t]
            a2 = cpool.tile([P, NE, T], f16, tag="a2")
            for e in range(NE):
                pa = ps_ta.tile([P, T], f32, tag="ta", name="pa2")
                for j in range(NT):
                    nc.tensor.matmul(
                        pa[:],
                        lhsT=x1n[b][:, j, e * P : (e + 1) * P],
                        rhs=alphaT[:, j, :],
                        start=(j == 0),
                        stop=(j == NT - 1),
                    )
                nc.vector.tensor_copy(out=a2[:, e, :], in_=pa[:])
            # o{1,2}pre^T[f, t] = sum_e U[e,f] x^T[e,t] + sum_e V[e,f] a^T[e,t]
            last_b = b == BL - 1
            for half, (wu, wv, xt_, at) in enumerate(
                (("u1", "v1", x1t[b], a1), ("u2", "v2", x2t[b], a2))
            ):
                for f in range(NE):
                    col = b * 8 + half * NE + f
                    if last_b and half == 1 and f == NE - 1:
                        # Final tile of the whole kernel: compute it as two
                        # half-width chains so the trailing tanh+accum only
                        # covers 256 columns -> shorter pipeline drain.
                        acc = []
                        for hf in range(2):
                            sl = slice(hf * (T // 2), (hf + 1) * (T // 2))
                            # S-ring is idle after S(BL-1): no rotation stalls
                            po = ps_s.tile([P, T // 2], f32, tag="s", name="poh")
                            for e in range(NE):
                                nc.tensor.matmul(
                                    po[:],
                                    lhsT=ws[wu][:, e, f * P : (f + 1) * P],
                                    rhs=xt_[:, e, sl],
                                    start=(e == 0),
                                    stop=False,
                                )
                            for e in range(NE):
                                nc.tensor.matmul(
                                    po[:],
                                    lhsT=ws[wv][:, e, f * P : (f + 1) * P],
                                    rhs=at[:, e, sl],
                                    start=False,
                                    stop=(e == NE - 1),
                                )
                            trash = tpool.tile([P, T // 2], f16, tag="trash_h")
                            ac = spool.tile([P, 1], f32, tag=f"acc{hf}")
                            nc.scalar.activation(
                                trash[:], po[:], AF.Tanh, accum_out=ac[:]
                            )
                            acc.append(ac)
                        nc.vector.tensor_tensor(
                            stage[:, col : col + 1],
                            acc[0][:],
                            acc[1][:],
                            mybir.AluOpType.add,
                        )
                        continue
                    po = ps_o.tile([P, T], f32, tag="o")
                    for e in range(NE):
                        nc.tensor.matmul(
                            po[:],
                            lhsT=ws[wu][:, e, f * P : (f + 1) * P],
                            rhs=xt_[:, e, :],
                            start=(e == 0),
                            stop=False,
                        )
                    for e in range(NE):
                        nc.tensor.matmul(
                            po[:],
                            lhsT=ws[wv][:, e, f * P : (f + 1) * P],
                            rhs=at[:, e, :],
                            start=False,
                            stop=(e == NE - 1),
                        )
                    trash = tpool.tile([P, T], f16, tag="trash")
                    nc.scalar.activation(
                        trash[:],
                        po[:],
                        AF.Tanh,
                        accum_out=stage[:, col : col + 1],
                    )

        # Software pipeline; PE stream per step b:
        #   S(b) | transpose_alpha(b-1) | rest(b-1)
        # (S before T: S(b) has no dependency on alpha(b-1), so it fills
        # the PE while batch b-1's softmax epilogue finishes.)
        prev = None
        for b in range(BL):
            alpha, alpha8 = s_phase(b)
            if prev is not None:
                prev_alphaT = transpose_alpha(prev[0])
                rest_phase(b - 1, prev[1], prev_alphaT)
            prev = (alpha, alpha8)
        prev_alphaT = transpose_alpha(prev[0])
        rest_phase(BL - 1, prev[1], prev_alphaT)

        # Final: stage^T @ (I/T) transposes AND applies the mean scale in
        # one matmul; then a DVE copy to SBUF and a single contiguous DMA.
        pfin = ps_o.tile([8 * BL, P], f32, tag="o", name="pfin")
        nc.tensor.matmul(
            pfin[:], lhsT=stage[:], rhs=id_f32s[:], start=True, stop=True
        )
        fin = tpool.tile([8 * BL, P], f32, tag="fin_sb")
        nc.vector.tensor_copy(out=fin[:], in_=pfin[:])
        nc.sync.dma_start(out_d.ap().rearrange("b (x f) -> (b x) f", f=P), fin[:])

    nc.compile()
    return nc


def _get_nc():
    if "nc" not in _CACHE:
        _CACHE["nc"] = _build()
    return _CACHE["nc"]


def _make_in_maps(inputs):
    import ml_dtypes

    f16 = np.float16
    f8 = ml_dtypes.float8_e4m3
    x1 = np.asarray(inputs["x1"], dtype=np.float32).astype(f16)
    x2 = np.asarray(inputs["x2"], dtype=np.float32).astype(f16)
    wmap = {
        nm: np.ascontiguousarray(np.asarray(inputs[NM], dtype=np.float32)).astype(f16)
        for nm, NM in (("u1", "U1"), ("v1", "V1"), ("u2", "U2"), ("v2", "V2"))
    }
    in_maps = []
    for c in range(NCORES):
        sl = slice(c * BL, (c + 1) * BL)
        m = {
            "x1": np.ascontiguousarray(x1[sl]),
            # a1's fp8 lhsT: quantize the fp16 copy (matches emulation)
            "x2": np.ascontiguousarray(x2[sl]).astype(f8),
            "x1t": np.ascontiguousarray(x1[sl].transpose(0, 2, 1)),
            "x2t": np.ascontiguousarray(x2[sl].transpose(0, 2, 1)),
        }
        m.update(wmap)
        in_maps.append(m)
    return in_maps


def _run(inputs, trace=False, **kw):
    from concourse.bass_utils import run_bass_kernel_spmd

    nc = _get_nc()
    res = run_bass_kernel_spmd(
        nc, _make_in_maps(inputs), core_ids=list(range(NCORES)), trace=trace, **kw
    )
    out = np.concatenate([r["out"] for r in res.results], axis=0)
    return np.asarray(out, dtype=np.float32), res


def kernel(**inputs):
    out, _ = _run(inputs, trace=False)
    return out


# revision 42
# speedup vs baseline: 1.0158x; 1.0158x over previous
"""Trainium2 Bass kernel for AttentionMM.

Reference computation (per batch b, T=E=512):
    alpha = softmax(x1 @ x2^T, axis=-1)              # [T, T]
    a1t   = alpha^T @ x2                             # [T, E]
    a2t   = alpha @ x1                               # [T, E]
    o1    = mean_t tanh(x1 @ U1 + a1t @ V1)          # [E]
    o2    = mean_t tanh(x2 @ U2 + a2t @ V2)          # [E]
    out[b] = concat(o1, o2)                          # [2E]

Sharding: data-parallel over batch across 8 NeuronCores (4 batches/core),
U1/U2/V1/V2 replicated.  No collectives needed; shard/gather on host.

Matmuls run in fp16 (full TensorEngine rate, ~8x better accuracy than
bf16) except a1's, which runs in fp8 e4m3 with DoubleRow (two 128-deep
k-tiles per matmul; alpha quantizes well, measured rel_err 1.47e-2 vs the
2e-2 gate, emulator-validated).  Accumulation is f32 in PSUM.  Per-batch
dataflow keeps everything in "transposed" layouts so that every
contraction lands on the partition axis and the final mean over T is a
free-axis reduction (done for free by activation(accum_out=...)):
    S      = x1 @ x2^T          via lhsT=x1^T blocks, rhs=x2^T   -> [t, s]
    alpha  = softmax rows (reduce_max(negate) -> Exp(bias=-max, accum_out)
             -> reciprocal -> tensor_scalar_mul)
    alphaT = PE-transpose of alpha blocks (16x 128x128)
    a1^T   = lhsT=x2 blocks,  rhs=alpha                          -> [e, t]
    a2^T   = lhsT=x1 blocks,  rhs=alphaT                         -> [e, t]
    o1pre^T= lhsT=U1 blocks, rhs=x1^T  (+) lhsT=V1 blocks, rhs=a1^T
    o2pre^T= lhsT=U2 blocks, rhs=x2^T  (+) lhsT=V2 blocks, rhs=a2^T
    tanh with accum_out -> per-partition sums -> staged, one final PE
    transpose + scale by 1/T -> single contiguous DMA out.

PSUM budget (8 banks): 3 for the S chains (i-outer, stall-free vs the
softmax consumer chain), 3 shared ring for alpha-transposes + a1/a2, 2 for
the o-phase.  PE warmups run on a zeroed tile with no data dependencies so
the HAM clock-gate lifts before the first real matmul.
"""

import sys

if "/opt/trn_rl_repo" not in sys.path:
    sys.path.insert(0, "/opt/trn_rl_repo")

import numpy as np

B, T, E = 32, 512, 512
NCORES = 8
BL = B // NCORES  # batches per core
P = 128
NT = T // P
NE = E // P

_CACHE = {}


def _build():
    from contextlib import ExitStack

    import concourse.bass as bass
    import concourse.tile as tile
    from concourse import bacc, mybir
    from concourse.masks import make_identity

    f16 = mybir.dt.float16
    f32 = mybir.dt.float32
    f8 = mybir.dt.float8e4
    DR = mybir.MatmulPerfMode.DoubleRow
    AF = mybir.ActivationFunctionType
    AX = mybir.AxisListType

    nc = bacc.Bacc(
        "TRN2",
        target_bir_lowering=False,
        debug=False,
        enable_asserts=False,
        num_devices=NCORES,
    )

    x1_d = nc.dram_tensor("x1", [BL, T, E], f16, kind="ExternalInput")
    # x2 in row-major layout only feeds a1's fp8 DoubleRow lhsT
    x2_d = nc.dram_tensor("x2", [BL, T, E], f8, kind="ExternalInput")
    # host-pretransposed copies: x1t[b, e, t] = x1[b, t, e]
    x1t_d = nc.dram_tensor("x1t", [BL, E, T], f16, kind="ExternalInput")
    x2t_d = nc.dram_tensor("x2t", [BL, E, T], f16, kind="ExternalInput")
    w_d = {
        nm: nc.dram_tensor(nm, [E, E], f16, kind="ExternalInput")
        for nm in ("u1", "v1", "u2", "v2")
    }
    out_d = nc.dram_tensor("out", [BL, 2 * E], f32, kind="ExternalOutput")

    with tile.TileContext(nc) as tc, ExitStack() as ctx:
        const = ctx.enter_context(tc.tile_pool(name="const", bufs=1))
        wpool = ctx.enter_context(tc.tile_pool(name="wts", bufs=1))
        xpool = ctx.enter_context(tc.tile_pool(name="x", bufs=BL))
        apool = ctx.enter_context(tc.tile_pool(name="alpha", bufs=2))
        cpool = ctx.enter_context(tc.tile_pool(name="attn", bufs=2))
        spool = ctx.enter_context(tc.tile_pool(name="stats", bufs=16))
        tpool = ctx.enter_context(tc.tile_pool(name="trash", bufs=2))
        stgp = ctx.enter_context(tc.tile_pool(name="stage", bufs=1))
        ps_s = ctx.enter_context(tc.tile_pool(name="ps_s", bufs=3, space="PSUM"))
        ps_ta = ctx.enter_context(tc.tile_pool(name="ps_ta", bufs=3, space="PSUM"))
        ps_o = ctx.enter_context(tc.tile_pool(name="ps_o", bufs=2, space="PSUM"))

        # Warmup operand: memset on the vector engine (whose preamble ends
        # earliest) so the warmup matmuls issue as soon as possible and
        # lift the HAM clock gate before the first data-dependent matmul.
        warm = const.tile([P, T], f16, tag="warm")
        nc.vector.memset(warm[:], 0)
        id_f16 = const.tile([P, P], f16, tag="id_f16")
        make_identity(nc, id_f16[:])
        # identity pre-scaled by 1/T: the final stage transpose then yields
        # the mean directly (one DVE copy instead of a scalar mul after)
        id_f32s = const.tile([P, P], f32, tag="id_f32s")
        make_identity(nc, id_f32s[:])
        nc.scalar.mul(id_f32s[:], id_f32s[:], 1.0 / T)

        # col = b*8 + half*4 + f  ->  out[b, half*512 + f*128 : +128]
        stage = stgp.tile([P, 8 * BL], f32, tag="stage")

        # ---- tiles (allocated up front so DMAs can be emitted in the
        # exact order they should occupy the single sync DMA ring) ----
        x1t = {}
        x2t = {}
        x1n = {}
        x2n = {}
        for b in range(BL):
            x1t[b] = xpool.tile([P, NE, T], f16, tag="x1t", name=f"x1t{b}")
            x2t[b] = xpool.tile([P, NE, T], f16, tag="x2t", name=f"x2t{b}")
            x1n[b] = xpool.tile([P, NT, E], f16, tag="x1n", name=f"x1n{b}")
            x2n[b] = xpool.tile([P, NT, E], f8, tag="x2n", name=f"x2n{b}")
        ws = {
            nm: wpool.tile([P, NE, E], f16, tag=nm, name=nm)
            for nm in ("u1", "v1", "u2", "v2")
        }

        def load_xt(b, chunked=False):
            if chunked:
                # interleave half-tensor slabs so the first S accumulation
                # chain can start after the first pair arrives (same-tile
                # chunk DMAs serialize on their completion semaphore, so
                # keep the chunk count low)
                H = NE // 2
                for h in range(2):
                    nc.sync.dma_start(
                        x1t[b][:, h * H : (h + 1) * H, :],
                        x1t_d.ap()[b, h * H * P : (h + 1) * H * P, :].rearrange(
                            "(a p) t -> p a t", p=P
                        ),
                    )
                    nc.sync.dma_start(
                        x2t[b][:, h * H : (h + 1) * H, :],
                        x2t_d.ap()[b, h * H * P : (h + 1) * H * P, :].rearrange(
                            "(a p) t -> p a t", p=P
                        ),
                    )
            else:
                nc.sync.dma_start(
                    x1t[b][:], x1t_d.ap()[b].rearrange("(a p) t -> p a t", p=P)
                )
                nc.sync.dma_start(
                    x2t[b][:], x2t_d.ap()[b].rearrange("(a p) t -> p a t", p=P)
                )

        def load_xn(b):
            # x2n first: it is a1's lhsT, the first consumer after softmax
            nc.sync.dma_start(
                x2n[b][:], x2_d.ap()[b].rearrange("(i p) e -> p i e", p=P)
            )
            nc.sync.dma_start(
                x1n[b][:], x1_d.ap()[b].rearrange("(i p) e -> p i e", p=P)
            )

        def load_w(nm):
            nc.sync.dma_start(ws[nm][:], w_d[nm].ap().rearrange("(a p) f -> p a f", p=P))

        # All loads on ONE queue (sync), in strict need-time order: a single
        # FIFO ring keeps early consumers' transfers from being bandwidth-
        # starved by later ones.
        load_xt(0, chunked=True)
        load_xt(1, chunked=True)
        load_xn(0)
        for nm in ("u1", "v1", "u2", "v2"):
            load_w(nm)
        load_xt(2)
        load_xn(1)
        load_xt(3)
        load_xn(2)
        load_xn(3)

        # PE warmups (HAM clock gate): no data deps, results discarded.
        # They rotate through the o-phase PSUM ring, which has no real
        # consumer until ~18us in, so they never delay the S chains.
        for k in range(8):
            warm_ps = ps_o.tile([P, T], f32, tag="o", name=f"warm{k}")
            nc.tensor.matmul(
                warm_ps[:], lhsT=warm[:, :P], rhs=warm[:], start=True, stop=True
            )

        def s_phase(b):
            """S = x1 @ x2^T, then row softmax -> alpha [t-part, s-free].
            Also materializes an fp8 copy of alpha for a1's DoubleRow mms."""
            alpha = apool.tile([P, NT, T], f16, tag="alpha")
            alpha8 = apool.tile([P, NT, T], f8, tag="alpha8")
            for i in range(NT):
                ps = ps_s.tile([P, T], f32, tag="s")
                for e in range(NE):
                    nc.tensor.matmul(
                        ps[:],
                        lhsT=x1t[b][:, e, i * P : (i + 1) * P],
                        rhs=x2t[b][:, e, :],
                        start=(e == 0),
                        stop=(e == NE - 1),
                    )
                mneg = spool.tile([P, 1], f32, tag="mneg")
                nc.vector.reduce_max(out=mneg[:], in_=ps[:], axis=AX.X, negate=True)
                ssum = spool.tile([P, 1], f32, tag="ssum")
                nc.scalar.activation(
                    alpha[:, i, :], ps[:], AF.Exp, bias=mneg[:], accum_out=ssum[:]
                )
                rcol = spool.tile([P, 1], f32, tag="rcol")
                nc.vector.reciprocal(rcol[:], ssum[:])
                nc.vector.tensor_scalar_mul(alpha[:, i, :], alpha[:, i, :], rcol[:])
                # NOTE: keep this cast on DVE -- GpSimd runs it 4.4x slower
                # (1.87us vs 0.43us per [128,512] tile) and serializes a1
                nc.vector.tensor_copy(out=alpha8[:, i, :], in_=alpha[:, i, :])
            return alpha, alpha8

        def transpose_alpha(alpha):
            """alphaT[j-part, t-free] via 16 PE block transposes, emitted
            ahead of the NEXT batch's S matmuls."""
            alphaT = apool.tile([P, NT, T], f16, tag="alphaT")
            for j in range(NT):
                pst = ps_ta.tile([P, T], f16, tag="ta", name="pst")
                for i in range(NT):
                    nc.tensor.transpose(
                        pst[:, i * P : (i + 1) * P],
                        alpha[:, i, j * P : (j + 1) * P],
                        id_f16[:],
                    )
                nc.vector.tensor_copy(out=alphaT[:, j, :], in_=pst[:])
            return alphaT

        def rest_phase(b, alpha8, alphaT):
            # a1^T[e, t] = sum_k x2[k, e] * alpha[k, t]  -- fp8 DoubleRow:
            # each matmul contracts two 128-deep k-tiles at once.
            a1 = cpool.tile([P, NE, T], f16, tag="a1")
            for e in range(NE):
                pa = ps_ta.tile([P, T], f32, tag="ta", name="pa1")
                for q in range(NT // 2):
                    nc.tensor.matmul(
                        pa[:],
                        lhsT=x2n[b][:, 2 * q : 2 * q + 2, e * P : (e + 1) * P],
                        rhs=alpha8[:, 2 * q : 2 * q + 2, :],
                        start=(q == 0),
                        stop=(q == NT // 2 - 1),
                        perf_mode=DR,
                    )
                nc.vector.tensor_copy(out=a1[:, e, :], in_=pa[:])
            # a2^T[e, t] = sum_s x1[s, e] * alphaT[s, t]
            a2 = cpool.tile([P, NE, T], f16, tag="a2")
            for e in range(NE):
                pa = ps_ta.tile([P, T], f32, tag="ta", name="pa2")
                for j in range(NT):
                    nc.tensor.matmul(
                        pa[:],
                        lhsT=x1n[b][:, j, e * P : (e + 1) * P],
                        rhs=alphaT[:, j, :],
                        start=(j == 0),
                        stop=(j == NT - 1),
                    )
                nc.vector.tensor_copy(out=a2[:, e, :], in_=pa[:])
            # o{1,2}pre^T[f, t] = sum_e U[e,f] x^T[e,t] + sum_e V[e,f] a^T[e,t]
            last_b = b == BL - 1
            for half, (wu, wv, xt_, at) in enumerate(
                (("u1", "v1", x1t[b], a1), ("u2", "v2", x2t[b], a2))
            ):
                for f in range(NE):
                    col = b * 8 + half * NE + f
                    if last_b and half == 1 and f == NE - 1:
                        # Final tile of the whole kernel: compute it as two
                        # half-width chains so the trailing tanh+accum only
                        # covers 256 columns -> shorter pipeline drain.
                        acc = []
                        for hf in range(2):
                            sl = slice(hf * (T // 2), (hf + 1) * (T // 2))
                            # S-ring is idle after S(BL-1): no rotation stalls
                            po = ps_s.tile([P, T // 2], f32, tag="s", name="poh")
                            for e in range(NE):
                                nc.tensor.matmul(
                                    po[:],
                                    lhsT=ws[wu][:, e, f * P : (f + 1) * P],
                                    rhs=xt_[:, e, sl],
                                    start=(e == 0),
                                    stop=False,
                                )
                            for e in range(NE):
                                nc.tensor.matmul(
                                    po[:],
                                    lhsT=ws[wv][:, e, f * P : (f + 1) * P],
                                    rhs=at[:, e, sl],
                                    start=False,
                                    stop=(e == NE - 1),
                                )
                            trash = tpool.tile([P, T // 2], f16, tag="trash_h")
                            ac = spool.tile([P, 1], f32, tag=f"acc{hf}")
                            nc.scalar.activation(
                                trash[:], po[:], AF.Tanh, accum_out=ac[:]
                            )
                            acc.append(ac)
                        nc.vector.tensor_tensor(
                            stage[:, col : col + 1],
                            acc[0][:],
                            acc[1][:],
                            mybir.AluOpType.add,
                        )
                        continue
                    po = ps_o.tile([P, T], f32, tag="o")
                    for e in range(NE):
                        nc.tensor.matmul(
                            po[:],
                            lhsT=ws[wu][:, e, f * P : (f + 1) * P],
                            rhs=xt_[:, e, :],
                            start=(e == 0),
                            stop=False,
                        )
                    for e in range(NE):
                        nc.tensor.matmul(
                            po[:],
                            lhsT=ws[wv][:, e, f * P : (f + 1) * P],
                            rhs=at[:, e, :],
                            start=False,
                            stop=(e == NE - 1),
                        )
                    trash = tpool.tile([P, T], f16, tag="trash")
                    nc.scalar.activation(
                        trash[:],
                        po[:],
                        AF.Tanh,
                        accum_out=stage[:, col : col + 1],
                    )

        # Software pipeline; PE stream per step b:
        #   S(b) | transpose_alpha(b-1) | rest(b-1)
        # (S before T: S(b) has no dependency on alpha(b-1), so it fills
        # the PE while batch b-1's softmax epilogue finishes.)
        prev = None
        for b in range(BL):
            alpha, alpha8 = s_phase(b)
            if prev is not None:
                prev_alphaT = transpose_alpha(prev[0])
                rest_phase(b - 1, prev[1], prev_alphaT)
            prev = (alpha, alpha8)
        prev_alphaT = transpose_alpha(prev[0])
        rest_phase(BL - 1, prev[1], prev_alphaT)

        # Final: stage^T @ (I/T) transposes AND applies the mean scale in
        # one matmul; then a DVE copy to SBUF and a single contiguous DMA.
        pfin = ps_o.tile([8 * BL, P], f32, tag="o", name="pfin")
        nc.tensor.matmul(
            pfin[:], lhsT=stage[:], rhs=id_f32s[:], start=True, stop=True
        )
        fin = tpool.tile([8 * BL, P], f32, tag="fin_sb")
        nc.vector.tensor_copy(out=fin[:], in_=pfin[:])
        nc.sync.dma_start(out_d.ap().rearrange("b (x f) -> (b x) f", f=P), fin[:])

    nc.compile()
    return nc


def _get_nc():
    if "nc" not in _CACHE:
        _CACHE["nc"] = _build()
    return _CACHE["nc"]


def _make_in_maps(inputs):
    import ml_dtypes

    f16 = np.float16
    f8 = ml_dtypes.float8_e4m3
    x1 = np.asarray(inputs["x1"], dtype=np.float32).astype(f16)
    x2 = np.asarray(inputs["x2"], dtype=np.float32).astype(f16)
    wmap = {
        nm: np.ascontiguousarray(np.asarray(inputs[NM], dtype=np.float32)).astype(f16)
        for nm, NM in (("u1", "U1"), ("v1", "V1"), ("u2", "U2"), ("v2", "V2"))
    }
    in_maps = []
    for c in range(NCORES):
        sl = slice(c * BL, (c + 1) * BL)
        m = {
            "x1": np.ascontiguousarray(x1[sl]),
            # a1's fp8 lhsT: quantize the fp16 copy (matches emulation)
            "x2": np.ascontiguousarray(x2[sl]).astype(f8),
            "x1t": np.ascontiguousarray(x1[sl].transpose(0, 2, 1)),
            "x2t": np.ascontiguousarray(x2[sl].transpose(0, 2, 1)),
        }
        m.update(wmap)
        in_maps.append(m)
    return in_maps


def _run(inputs, trace=False, **kw):
    from concourse.bass_utils import run_bass_kernel_spmd

    nc = _get_nc()
    res = run_bass_kernel_spmd(
        nc, _make_in_maps(inputs), core_ids=list(range(NCORES)), trace=trace, **kw
    )
    out = np.concatenate([r["out"] for r in res.results], axis=0)
    return np.asarray(out, dtype=np.float32), res


def kernel(**inputs):
    out, _ = _run(inputs, trace=False)
    return out


# revision 49
# speedup vs baseline: 1.0170x; 1.0011x over previous
"""Trainium2 Bass kernel for AttentionMM.

Reference computation (per batch b, T=E=512):
    alpha = softmax(x1 @ x2^T, axis=-1)              # [T, T]
    a1t   = alpha^T @ x2                             # [T, E]
    a2t   = alpha @ x1                               # [T, E]
    o1    = mean_t tanh(x1 @ U1 + a1t @ V1)          # [E]
    o2    = mean_t tanh(x2 @ U2 + a2t @ V2)          # [E]
    out[b] = concat(o1, o2)                          # [2E]

Sharding: data-parallel over batch across 8 NeuronCores (4 batches/core),
U1/U2/V1/V2 replicated.  No collectives needed; shard/gather on host.

Matmuls run in fp16 (full TensorEngine rate, ~8x better accuracy than
bf16) except a1's, which runs in fp8 e4m3 with DoubleRow (two 128-deep
k-tiles per matmul; alpha quantizes well, measured rel_err 1.47e-2 vs the
2e-2 gate, emulator-validated).  Accumulation is f32 in PSUM.  Per-batch
dataflow keeps everything in "transposed" layouts so that every
contraction lands on the partition axis and the final mean over T is a
free-axis reduction (done for free by activation(accum_out=...)):
    S      = x1 @ x2^T          via lhsT=x1^T blocks, rhs=x2^T   -> [t, s]
    alpha  = softmax rows (reduce_max(negate) -> Exp(bias=-max, accum_out)
             -> reciprocal -> tensor_scalar_mul)
    alphaT = PE-transpose of alpha blocks (16x 128x128)
    a1^T   = lhsT=x2 blocks,  rhs=alpha                          -> [e, t]
    a2^T   = lhsT=x1 blocks,  rhs=alphaT                         -> [e, t]
    o1pre^T= lhsT=U1 blocks, rhs=x1^T  (+) lhsT=V1 blocks, rhs=a1^T
    o2pre^T= lhsT=U2 blocks, rhs=x2^T  (+) lhsT=V2 blocks, rhs=a2^T
    tanh with accum_out -> per-partition sums -> staged, one final PE
    transpose + scale by 1/T -> single contiguous DMA out.

PSUM budget (8 banks): 3 for the S chains (i-outer, stall-free vs the
softmax consumer chain), 3 shared ring for alpha-transposes + a1/a2, 2 for
the o-phase.  PE warmups run on a zeroed tile with no data dependencies so
the HAM clock-gate lifts before the first real matmul.
"""

import sys

if "/opt/trn_rl_repo" not in sys.path:
    sys.path.insert(0, "/opt/trn_rl_repo")

import numpy as np

B, T, E = 32, 512, 512
NCORES = 8
BL = B // NCORES  # batches per core
P = 128
NT = T // P
NE = E // P

_CACHE = {}


def _build():
    from contextlib import ExitStack

    import concourse.bass as bass
    import concourse.tile as tile
    from concourse import bacc, mybir
    from concourse.masks import make_identity

    f16 = mybir.dt.float16
    f32 = mybir.dt.float32
    f8 = mybir.dt.float8e4
    DR = mybir.MatmulPerfMode.DoubleRow
    AF = mybir.ActivationFunctionType
    AX = mybir.AxisListType

    nc = bacc.Bacc(
        "TRN2",
        target_bir_lowering=False,
        debug=False,
        enable_asserts=False,
        num_devices=NCORES,
    )

    x1_d = nc.dram_tensor("x1", [BL, T, E], f16, kind="ExternalInput")
    # x2 in row-major layout only feeds a1's fp8 DoubleRow lhsT
    x2_d = nc.dram_tensor("x2", [BL, T, E], f8, kind="ExternalInput")
    # batch 0's a1 runs in fp16 instead (its DoubleRow weight loads contend
    # with in-flight input DMAs and the alpha8 cast chain) -> f16 copy
    x2f_d = nc.dram_tensor("x2f", [T, E], f16, kind="ExternalInput")
    # host-pretransposed copies: x1t[b, e, t] = x1[b, t, e]
    x1t_d = nc.dram_tensor("x1t", [BL, E, T], f16, kind="ExternalInput")
    x2t_d = nc.dram_tensor("x2t", [BL, E, T], f16, kind="ExternalInput")
    w_d = {
        nm: nc.dram_tensor(nm, [E, E], f16, kind="ExternalInput")
        for nm in ("u1", "v1", "u2", "v2")
    }
    out_d = nc.dram_tensor("out", [BL, 2 * E], f32, kind="ExternalOutput")

    with tile.TileContext(nc) as tc, ExitStack() as ctx:
        const = ctx.enter_context(tc.tile_pool(name="const", bufs=1))
        wpool = ctx.enter_context(tc.tile_pool(name="wts", bufs=1))
        xpool = ctx.enter_context(tc.tile_pool(name="x", bufs=BL))
        apool = ctx.enter_context(tc.tile_pool(name="alpha", bufs=2))
        cpool = ctx.enter_context(tc.tile_pool(name="attn", bufs=2))
        spool = ctx.enter_context(tc.tile_pool(name="stats", bufs=16))
        tpool = ctx.enter_context(tc.tile_pool(name="trash", bufs=2))
        stgp = ctx.enter_context(tc.tile_pool(name="stage", bufs=1))
        ps_s = ctx.enter_context(tc.tile_pool(name="ps_s", bufs=3, space="PSUM"))
        ps_ta = ctx.enter_context(tc.tile_pool(name="ps_ta", bufs=3, space="PSUM"))
        ps_o = ctx.enter_context(tc.tile_pool(name="ps_o", bufs=2, space="PSUM"))

        # Warmup operand: memset on the vector engine (whose preamble ends
        # earliest) so the warmup matmuls issue as soon as possible and
        # lift the HAM clock gate before the first data-dependent matmul.
        warm = const.tile([P, T], f16, tag="warm")
        nc.vector.memset(warm[:], 0)
        id_f16 = const.tile([P, P], f16, tag="id_f16")
        make_identity(nc, id_f16[:])
        # identity pre-scaled by 1/T: the final stage transpose then yields
        # the mean directly (one DVE copy instead of a scalar mul after)
        id_f32s = const.tile([P, P], f32, tag="id_f32s")
        make_identity(nc, id_f32s[:])
        nc.scalar.mul(id_f32s[:], id_f32s[:], 1.0 / T)

        # col = b*8 + half*4 + f  ->  out[b, half*512 + f*128 : +128]
        stage = stgp.tile([P, 8 * BL], f32, tag="stage")

        # ---- tiles (allocated up front so DMAs can be emitted in the
        # exact order they should occupy the single sync DMA ring) ----
        x1t = {}
        x2t = {}
        x1n = {}
        x2n = {}
        for b in range(BL):
            x1t[b] = xpool.tile([P, NE, T], f16, tag="x1t", name=f"x1t{b}")
            x2t[b] = xpool.tile([P, NE, T], f16, tag="x2t", name=f"x2t{b}")
            x1n[b] = xpool.tile([P, NT, E], f16, tag="x1n", name=f"x1n{b}")
            if b == 0:
                x2n[b] = xpool.tile([P, NT, E], f16, tag="x2n0f", name="x2n0f")
            else:
                x2n[b] = xpool.tile([P, NT, E], f8, tag="x2n", name=f"x2n{b}")
        ws = {
            nm: wpool.tile([P, NE, E], f16, tag=nm, name=nm)
            for nm in ("u1", "v1", "u2", "v2")
        }

        def load_xt(b, chunked=False):
            if chunked:
                # interleave half-tensor slabs so the first S accumulation
                # chain can start after the first pair arrives (same-tile
                # chunk DMAs serialize on their completion semaphore, so
                # keep the chunk count low)
                H = NE // 2
                for h in range(2):
                    nc.sync.dma_start(
                        x1t[b][:, h * H : (h + 1) * H, :],
                        x1t_d.ap()[b, h * H * P : (h + 1) * H * P, :].rearrange(
                            "(a p) t -> p a t", p=P
                        ),
                    )
                    nc.sync.dma_start(
                        x2t[b][:, h * H : (h + 1) * H, :],
                        x2t_d.ap()[b, h * H * P : (h + 1) * H * P, :].rearrange(
                            "(a p) t -> p a t", p=P
                        ),
                    )
            else:
                nc.sync.dma_start(
                    x1t[b][:], x1t_d.ap()[b].rearrange("(a p) t -> p a t", p=P)
                )
                nc.sync.dma_start(
                    x2t[b][:], x2t_d.ap()[b].rearrange("(a p) t -> p a t", p=P)
                )

        def load_xn(b):
            # x2n first: it is a1's lhsT, the first consumer after softmax
            if b == 0:
                nc.sync.dma_start(
                    x2n[b][:], x2f_d.ap().rearrange("(i p) e -> p i e", p=P)
                )
            else:
                nc.sync.dma_start(
                    x2n[b][:], x2_d.ap()[b].rearrange("(i p) e -> p i e", p=P)
                )
            nc.sync.dma_start(
                x1n[b][:], x1_d.ap()[b].rearrange("(i p) e -> p i e", p=P)
            )

        def load_w(nm):
            nc.sync.dma_start(ws[nm][:], w_d[nm].ap().rearrange("(a p) f -> p a f", p=P))

        # All loads on ONE queue (sync), in strict need-time order: a single
        # FIFO ring keeps early consumers' transfers from being bandwidth-
        # starved by later ones.
        load_xt(0, chunked=True)
        load_xt(1, chunked=True)
        load_xn(0)
        for nm in ("u1", "v1", "u2", "v2"):
            load_w(nm)
        load_xt(2)
        load_xn(1)
        load_xt(3)
        load_xn(2)
        load_xn(3)

        # PE warmups (HAM clock gate): no data deps, results discarded.
        # They rotate through the o-phase PSUM ring, which has no real
        # consumer until ~18us in, so they never delay the S chains.
        for k in range(8):
            warm_ps = ps_o.tile([P, T], f32, tag="o", name=f"warm{k}")
            nc.tensor.matmul(
                warm_ps[:], lhsT=warm[:, :P], rhs=warm[:], start=True, stop=True
            )

        def s_phase(b):
            """S = x1 @ x2^T, then row softmax -> alpha [t-part, s-free].
            Also materializes an fp8 copy of alpha for a1's DoubleRow mms."""
            alpha = apool.tile([P, NT, T], f16, tag="alpha")
            alpha8 = apool.tile([P, NT, T], f8, tag="alpha8")
            for i in range(NT):
                ps = ps_s.tile([P, T], f32, tag="s")
                for e in range(NE):
                    nc.tensor.matmul(
                        ps[:],
                        lhsT=x1t[b][:, e, i * P : (i + 1) * P],
                        rhs=x2t[b][:, e, :],
                        start=(e == 0),
                        stop=(e == NE - 1),
                    )
                mneg = spool.tile([P, 1], f32, tag="mneg")
                nc.vector.reduce_max(out=mneg[:], in_=ps[:], axis=AX.X, negate=True)
                ssum = spool.tile([P, 1], f32, tag="ssum")
                nc.scalar.activation(
                    alpha[:, i, :], ps[:], AF.Exp, bias=mneg[:], accum_out=ssum[:]
                )
                rcol = spool.tile([P, 1], f32, tag="rcol")
                nc.vector.reciprocal(rcol[:], ssum[:])
                nc.vector.tensor_scalar_mul(alpha[:, i, :], alpha[:, i, :], rcol[:])
                # NOTE: keep this cast on DVE -- GpSimd runs it 4.4x slower
                # (1.87us vs 0.43us per [128,512] tile) and a Scalar Copy
                # costs 810ns and makes Scalar the pacing engine.
                if b > 0:
                    nc.vector.tensor_copy(out=alpha8[:, i, :], in_=alpha[:, i, :])
            return alpha, alpha8

        def transpose_alpha(alpha):
            """alphaT[j-part, t-free] via 16 PE block transposes, emitted
            ahead of the NEXT batch's S matmuls."""
            alphaT = apool.tile([P, NT, T], f16, tag="alphaT")
            for j in range(NT):
                pst = ps_ta.tile([P, T], f16, tag="ta", name="pst")
                for i in range(NT):
                    nc.tensor.transpose(
                        pst[:, i * P : (i + 1) * P],
                        alpha[:, i, j * P : (j + 1) * P],
                        id_f16[:],
                    )
                nc.vector.tensor_copy(out=alphaT[:, j, :], in_=pst[:])
            return alphaT

        def rest_phase(b, alpha, alpha8, alphaT):
            # a1^T[e, t] = sum_k x2[k, e] * alpha[k, t]  -- fp8 DoubleRow
            # (two 128-deep k-tiles per matmul), except batch 0 where the
            # DR weight loads contend with in-flight input DMAs: plain fp16.
            a1 = cpool.tile([P, NE, T], f16, tag="a1")
            for e in range(NE):
                pa = ps_ta.tile([P, T], f32, tag="ta", name="pa1")
                if b == 0:
                    for i in range(NT):
                        nc.tensor.matmul(
                            pa[:],
                            lhsT=x2n[b][:, i, e * P : (e + 1) * P],
                            rhs=alpha[:, i, :],
                            start=(i == 0),
                            stop=(i == NT - 1),
                        )
                else:
                    for q in range(NT // 2):
                        nc.tensor.matmul(
                            pa[:],
                            lhsT=x2n[b][:, 2 * q : 2 * q + 2, e * P : (e + 1) * P],
                            rhs=alpha8[:, 2 * q : 2 * q + 2, :],
                            start=(q == 0),
                            stop=(q == NT // 2 - 1),
                            perf_mode=DR,
                        )
                nc.vector.tensor_copy(out=a1[:, e, :], in_=pa[:])
            # a2^T[e, t] = sum_s x1[s, e] * alphaT[s, t]
            a2 = cpool.tile([P, NE, T], f16, tag="a2")
            for e in range(NE):
                pa = ps_ta.tile([P, T], f32, tag="ta", name="pa2")
                for j in range(NT):
                    nc.tensor.matmul(
                        pa[:],
                        lhsT=x1n[b][:, j, e * P : (e + 1) * P],
                        rhs=alphaT[:, j, :],
                        start=(j == 0),
                        stop=(j == NT - 1),
                    )
                nc.vector.tensor_copy(out=a2[:, e, :], in_=pa[:])
            # o{1,2}pre^T[f, t] = sum_e U[e,f] x^T[e,t] + sum_e V[e,f] a^T[e,t]
            last_b = b == BL - 1
            for half, (wu, wv, xt_, at) in enumerate(
                (("u1", "v1", x1t[b], a1), ("u2", "v2", x2t[b], a2))
            ):
                for f in range(NE):
                    col = b * 8 + half * NE + f
                    if last_b and half == 1 and f == NE - 1:
                        # Final tile of the whole kernel: compute it as two
                        # half-width chains so the trailing tanh+accum only
                        # covers 256 columns -> shorter pipeline drain.
                        acc = []
                        for hf in range(2):
                            sl = slice(hf * (T // 2), (hf + 1) * (T // 2))
                            # S-ring is idle after S(BL-1): no rotation stalls
                            po = ps_s.tile([P, T // 2], f32, tag="s", name="poh")
                            for e in range(NE):
                                nc.tensor.matmul(
                                    po[:],
                                    lhsT=ws[wu][:, e, f * P : (f + 1) * P],
                                    rhs=xt_[:, e, sl],
                                    start=(e == 0),
                                    stop=False,
                                )
                            for e in range(NE):
                                nc.tensor.matmul(
                                    po[:],
                                    lhsT=ws[wv][:, e, f * P : (f + 1) * P],
                                    rhs=at[:, e, sl],
                                    start=False,
                                    stop=(e == NE - 1),
                                )
                            trash = tpool.tile([P, T // 2], f16, tag="trash_h")
                            ac = spool.tile([P, 1], f32, tag=f"acc{hf}")
                            nc.scalar.activation(
                                trash[:], po[:], AF.Tanh, accum_out=ac[:]
                            )
                            acc.append(ac)
                        nc.vector.tensor_tensor(
                            stage[:, col : col + 1],
                            acc[0][:],
                            acc[1][:],
                            mybir.AluOpType.add,
                        )
                        continue
                    po = ps_o.tile([P, T], f32, tag="o")
                    for e in range(NE):
                        nc.tensor.matmul(
                            po[:],
                            lhsT=ws[wu][:, e, f * P : (f + 1) * P],
                            rhs=xt_[:, e, :],
                            start=(e == 0),
                            stop=False,
                        )
                    for e in range(NE):
                        nc.tensor.matmul(
                            po[:],
                            lhsT=ws[wv][:, e, f * P : (f + 1) * P],
                            rhs=at[:, e, :],
                            start=False,
                            stop=(e == NE - 1),
                        )
                    trash = tpool.tile([P, T], f16, tag="trash")
                    nc.scalar.activation(
                        trash[:],
                        po[:],
                        AF.Tanh,
                        accum_out=stage[:, col : col + 1],
                    )

        # Software pipeline; PE stream per step b:
        #   S(b) | transpose_alpha(b-1) | rest(b-1)
        # (S before T: S(b) has no dependency on alpha(b-1), so it fills
        # the PE while batch b-1's softmax epilogue finishes.)
        prev = None
        for b in range(BL):
            alpha, alpha8 = s_phase(b)
            if prev is not None:
                prev_alphaT = transpose_alpha(prev[0])
                rest_phase(b - 1, prev[0], prev[1], prev_alphaT)
            prev = (alpha, alpha8)
        prev_alphaT = transpose_alpha(prev[0])
        rest_phase(BL - 1, prev[0], prev[1], prev_alphaT)

        # Final: stage^T @ (I/T) transposes AND applies the mean scale in
        # one matmul; then a DVE copy to SBUF and a single contiguous DMA.
        pfin = ps_o.tile([8 * BL, P], f32, tag="o", name="pfin")
        nc.tensor.matmul(
            pfin[:], lhsT=stage[:], rhs=id_f32s[:], start=True, stop=True
        )
        fin = tpool.tile([8 * BL, P], f32, tag="fin_sb")
        nc.vector.tensor_copy(out=fin[:], in_=pfin[:])
        nc.sync.dma_start(out_d.ap().rearrange("b (x f) -> (b x) f", f=P), fin[:])

    nc.compile()
    return nc


def _get_nc():
    if "nc" not in _CACHE:
        _CACHE["nc"] = _build()
    return _CACHE["nc"]


def _make_in_maps(inputs):
    import ml_dtypes

    f16 = np.float16
    f8 = ml_dtypes.float8_e4m3
    x1 = np.asarray(inputs["x1"], dtype=np.float32).astype(f16)
    x2 = np.asarray(inputs["x2"], dtype=np.float32).astype(f16)
    wmap = {
        nm: np.ascontiguousarray(np.asarray(inputs[NM], dtype=np.float32)).astype(f16)
        for nm, NM in (("u1", "U1"), ("v1", "V1"), ("u2", "U2"), ("v2", "V2"))
    }
    in_maps = []
    for c in range(NCORES):
        sl = slice(c * BL, (c + 1) * BL)
        m = {
            "x1": np.ascontiguousarray(x1[sl]),
            # a1's fp8 lhsT: quantize the fp16 copy (matches emulation)
            "x2": np.ascontiguousarray(x2[sl]).astype(f8),
            # fp16 copy of batch 0's x2 rows (its a1 runs in fp16)
            "x2f": np.ascontiguousarray(x2[sl][0]),
            "x1t": np.ascontiguousarray(x1[sl].transpose(0, 2, 1)),
            "x2t": np.ascontiguousarray(x2[sl].transpose(0, 2, 1)),
        }
        m.update(wmap)
        in_maps.append(m)
    return in_maps


def _run(inputs, trace=False, **kw):
    from concourse.bass_utils import run_bass_kernel_spmd

    nc = _get_nc()
    res = run_bass_kernel_spmd(
        nc, _make_in_maps(inputs), core_ids=list(range(NCORES)), trace=trace, **kw
    )
    out = np.concatenate([r["out"] for r in res.results], axis=0)
    return np.asarray(out, dtype=np.float32), res


def kernel(**inputs):
    out, _ = _run(inputs, trace=False)
    return out
